# revision 17
# baseline (speedup 1.0000x reference)
"""Multi-head self-attention (LayerNorm + causal MHA + output projection) on 8 trn2 cores.

Sharding: core i handles (batch b = i//2, head-group g = i%2); each head-group
is 8 of the 16 heads (512 of 1024 feature dims). The output projection is
row-sharded: each core computes a full [1024, 2048] partial; the host sums the
two partials per batch (the "all-reduce") and adds the output bias.

On-chip layout (per core), everything transposed so the contraction dim sits on
partitions:
  xnT  [c=128x8, s=2048]  bf16   LayerNorm'd input, transposed via PE
  qT,kT [o=128x4, s=2048] bf16   Q^T (pre-scaled by 1/sqrt(dh)), K^T
  v    [s=128x16, 8*(64+1)] bf16 V in [s, d] layout + ones column per head
                                 (the ones column makes the AV matmul also
                                  produce the softmax denominator row)
  scoresT psum [k=128, q]  f32   -> exp on ACT -> pT bf16 (causal blocks only)
  av psum [65, 512]        f32   rows 0..63 = (P@V)^T head slice, row 64 = l
  mergedT [m=128x4, s=2048] bf16 (P@V)^T / l, heads stacked
  out  [o=1024, s=2048]    f32   woT @ mergedT partial, DMA'd out
"""

import numpy as np
import ml_dtypes
from contextlib import ExitStack

import concourse.bass as bass
import concourse.mybir as mybir
import concourse.tile as tile
from concourse import bacc
from concourse.bass_utils import run_bass_kernel_spmd

f32 = mybir.dt.float32
bf16 = mybir.dt.bfloat16
AF = mybir.ActivationFunctionType
ALU = mybir.AluOpType

B, S, D = 4, 2048, 1024
H, DH = 16, 64
N_CORES = 8
O = 512          # head dims per core (8 heads)
HL = 8           # heads per core
CC = 8           # c chunks (1024/128)
ST = 16          # s tiles of 128
QT = 4           # q tiles of 512
EPS = 1e-5


def emit(nc, tc, ctx):
    x_d = nc.dram_tensor("x", [S, D], bf16, kind="ExternalInput").ap()
    wq_d = nc.dram_tensor("wq", [128, CC, O], bf16, kind="ExternalInput").ap()
    wk_d = nc.dram_tensor("wk", [128, CC, O], bf16, kind="ExternalInput").ap()
    wv_d = nc.dram_tensor("wv", [128, CC, O], bf16, kind="ExternalInput").ap()
    wo_d = nc.dram_tensor("wo", [128, 4, D], bf16, kind="ExternalInput").ap()
    g2_d = nc.dram_tensor("g2", [128, CC], f32, kind="ExternalInput").ap()
    b2_d = nc.dram_tensor("b2", [128, CC], f32, kind="ExternalInput").ap()
    bq_d = nc.dram_tensor("bq2", [128, 4], f32, kind="ExternalInput").ap()
    bk_d = nc.dram_tensor("bk2", [128, 4], f32, kind="ExternalInput").ap()
    bvb_d = nc.dram_tensor("bvb", [128, O], f32, kind="ExternalInput").ap()
    tri_d = nc.dram_tensor("tri", [128, 128], bf16, kind="ExternalInput").ap()
    id_d = nc.dram_tensor("ident", [128, 128], bf16, kind="ExternalInput").ap()
    out_d = nc.dram_tensor("out", [D, S], f32, kind="ExternalOutput").ap()

    persist = ctx.enter_context(tc.tile_pool(name="persist", bufs=1))

    xnT = persist.tile([128, CC, S], bf16)
    qT = persist.tile([128, 4, S], bf16)
    kT = persist.tile([128, 4, S], bf16)
    v_sb = persist.tile([128, ST, HL * 65], bf16)
    mergedT = persist.tile([128, 4, S], bf16)

    wq_sb = persist.tile([128, CC, O], bf16)
    wk_sb = persist.tile([128, CC, O], bf16)
    wv_sb = persist.tile([128, CC, O], bf16)
    wo_sb = persist.tile([128, 4, D], bf16)
    g2_sb = persist.tile([128, CC], f32)
    b2_sb = persist.tile([128, CC], f32)
    bq_sb = persist.tile([128, 4], f32)
    bk_sb = persist.tile([128, 4], f32)
    bvb_sb = persist.tile([128, O], f32)
    tri_sb = persist.tile([128, 128], bf16)
    id_sb = persist.tile([128, 128], bf16)
    eps_sb = persist.tile([128, 1], f32)

    # small constants first; the x tiles stream in during stage A; weights
    # queue behind them (sync DMA queue is in-order) and arrive before stage B
    for dst, src in [(id_sb, id_d), (g2_sb, g2_d), (b2_sb, b2_d)]:
        nc.sync.dma_start(out=dst, in_=src)
    nc.vector.memset(eps_sb, EPS)
    # warm the ACT function tables before the LN chain needs them
    warm_sb = persist.tile([128, 1], f32)
    nc.scalar.activation(out=warm_sb, in_=eps_sb, func=AF.Sqrt)
    nc.scalar.activation(out=warm_sb, in_=eps_sb, func=AF.Exp)
    # ones columns of v (col 64 of each 65-wide head block)
    v4 = v_sb.rearrange("p st (h e) -> p st h e", e=65)
    nc.vector.memset(v4[:, :, :, 64:65], 1.0)

    # ---- Stage A: LayerNorm + transpose -> xnT, V projection interleaved ----
    xpool = ctx.enter_context(tc.tile_pool(name="xpool", bufs=5))
    xnpool = ctx.enter_context(tc.tile_pool(name="xnpool", bufs=8))
    spool = ctx.enter_context(tc.tile_pool(name="spool", bufs=8))

    # weights go on the scalar engine's DMA queue so the x tiles (sync queue)
    # are not blocked behind them at startup
    for dst, src_ in [(wv_sb, wv_d), (bvb_sb, bvb_d), (wq_sb, wq_d), (bq_sb, bq_d),
                      (wk_sb, wk_d), (bk_sb, bk_d), (tri_sb, tri_d), (wo_sb, wo_d)]:
        nc.scalar.dma_start(out=dst, in_=src_)
    bvb_v = bvb_sb.rearrange("p (h e) -> p h e", e=64)

    with tc.tile_pool(name="trpsum", bufs=3, space="PSUM") as trpsum, \
         tc.tile_pool(name="vpsum", bufs=3, space="PSUM") as vpsum:
        for stq in range(4):
            xn_grp = []
            for j in range(4):
                st = stq * 4 + j
                x_t = xpool.tile([128, D], bf16, tag="x_t")
                nc.sync.dma_start(out=x_t[:, 0:512],
                                  in_=x_d[st * 128:(st + 1) * 128, 0:512])
                nc.sync.dma_start(out=x_t[:, 512:1024],
                                  in_=x_d[st * 128:(st + 1) * 128, 512:1024])
                stats = spool.tile([128, 2, 6], f32, tag="stats")
                nc.vector.bn_stats(out=stats[:, 0, :], in_=x_t[:, 0:512])
                nc.vector.bn_stats(out=stats[:, 1, :], in_=x_t[:, 512:1024])
                mv = spool.tile([128, 2], f32, tag="mv")
                nc.vector.bn_aggr(out=mv, in_=stats)
                sq = spool.tile([128, 1], f32, tag="sq")
                nc.scalar.activation(out=sq, in_=mv[:, 1:2], func=AF.Sqrt, bias=eps_sb)
                rstd = spool.tile([128, 1], f32, tag="rstd")
                nc.vector.reciprocal(rstd, sq)
                nmr = spool.tile([128, 1], f32, tag="nmr")
                nc.vector.scalar_tensor_tensor(
                    out=nmr, in0=mv[:, 0:1], scalar=-1.0, in1=rstd,
                    op0=ALU.mult, op1=ALU.mult)
                xn_t = xnpool.tile([128, D], bf16, tag="xn_t")
                nc.scalar.activation(out=xn_t, in_=x_t, func=AF.Identity,
                                     bias=nmr, scale=rstd)
                xn_grp.append(xn_t)
            for cc in range(CC):
                ps_tr = trpsum.tile([128, 512], bf16, tag="ps_tr")
                for j in range(4):
                    nc.tensor.transpose(ps_tr[:, j * 128:(j + 1) * 128],
                                        xn_grp[j][:, cc * 128:(cc + 1) * 128], id_sb)
                nc.scalar.activation(
                    out=xnT[:, cc, stq * 512:(stq + 1) * 512], in_=ps_tr,
                    func=AF.Identity, scale=g2_sb[:, cc:cc + 1],
                    bias=b2_sb[:, cc:cc + 1])
            # V projection for this stq group (k rows stq*512 .. stq*512+512)
            for j in range(4):
                st = stq * 4 + j
                ps = vpsum.tile([128, 512], f32, tag="ps_v")
                for cc in range(CC):
                    nc.tensor.matmul(ps, lhsT=xnT[:, cc, st * 128:(st + 1) * 128],
                                     rhs=wv_sb[:, cc, :],
                                     start=(cc == 0), stop=(cc == CC - 1))
                nc.vector.scalar_tensor_tensor(
                    out=v4[:, st, :, 0:64],
                    in0=ps.rearrange("p (h e) -> p h e", e=64),
                    scalar=1.0, in1=bvb_v, op0=ALU.mult, op1=ALU.add)

    # ---- Stage C: QK projection per q/k-slice + attention rounds ----
    ppool = ctx.enter_context(tc.tile_pool(name="ppool", bufs=6))
    rpool = ctx.enter_context(tc.tile_pool(name="rpool", bufs=3))
    ldram = ctx.enter_context(tc.tile_pool(name="ldram", bufs=4, space="DRAM"))
    opool = ctx.enter_context(tc.tile_pool(name="opool", bufs=4))

    # qt-major rounds: round qt only needs Q/K columns for s < (qt+1)*512, so
    # the QK projection for slice nt is emitted inside round nt-1 (PE filler
    # while ACT runs exp), and the output projection for columns of round qt-1
    # interleaves likewise.  This keeps the PE dense so HAM stays unthrottled.
    with tc.tile_pool(name="spsum", bufs=2, space="PSUM") as spsum, \
         tc.tile_pool(name="avpsum", bufs=3, space="PSUM") as avpsum, \
         tc.tile_pool(name="qkpsum", bufs=1, space="PSUM") as qkpsum:

        def qkproj(nt):
            for w_sb, dst, b_sb in [(wq_sb, qT, bq_sb), (wk_sb, kT, bk_sb)]:
                for oc in range(4):
                    ps = qkpsum.tile([128, 512], f32, tag="ps_qk")
                    for cc in range(CC):
                        nc.tensor.matmul(
                            ps, lhsT=w_sb[:, cc, oc * 128:(oc + 1) * 128],
                            rhs=xnT[:, cc, nt * 512:(nt + 1) * 512],
                            start=(cc == 0), stop=(cc == CC - 1))
                    nc.vector.tensor_scalar_add(
                        out=dst[:, oc, nt * 512:(nt + 1) * 512], in0=ps,
                        scalar1=b_sb[:, oc:oc + 1])

        def outproj(nt):
            for oc8 in range(8):
                ps_o = avpsum.tile([128, 512], f32, tag="ps_av")
                for mc in range(4):
                    nc.tensor.matmul(
                        ps_o[0:128, :], lhsT=wo_sb[:, mc, oc8 * 128:(oc8 + 1) * 128],
                        rhs=mergedT[:, mc, nt * 512:(nt + 1) * 512],
                        start=(mc == 0), stop=(mc == 3))
                ob = opool.tile([128, 512], f32, tag="ob")
                nc.vector.tensor_copy(ob, ps_o[0:128, :])
                nc.sync.dma_start(
                    out=out_d[oc8 * 128:(oc8 + 1) * 128, nt * 512:(nt + 1) * 512],
                    in_=ob)

        qkproj(0)
        for qt in range(QT):
            q0 = qt * 512
            nkc = 4 * qt + 4
            for h in range(HL):
                hr = 64 * (h % 2)
                hc = h // 2
                ql = qT[hr:hr + 64, hc, :]
                kl = kT[hr:hr + 64, hc, :]
                ps_av = avpsum.tile([128, 512], f32, tag="ps_av")
                for kc2 in range(nkc // 2):
                    ps_s = spsum.tile([128, 1024], f32, tag="ps_s")
                    pT = ppool.tile([128, 1024], bf16, tag="pT")
                    subs = []
                    for j in range(2):
                        kc = 2 * kc2 + j
                        dq = max(0, (kc - 4 * qt) * 128)
                        nc.tensor.matmul(
                            ps_s[:, j * 512 + dq:(j + 1) * 512],
                            lhsT=kl[:, kc * 128:(kc + 1) * 128],
                            rhs=ql[:, q0 + dq:q0 + 512], start=True, stop=True)
                        subs.append((j, kc, dq))
                    # one wide exp per psum group; unwritten prefix columns of
                    # diagonal blocks hold stale psum - exp'd but never read
                    nc.scalar.activation(out=pT, in_=ps_s, func=AF.Exp)
                    for j, kc, dq in subs:
                        if kc >= 4 * qt:
                            nc.vector.tensor_mul(
                                pT[:, j * 512 + dq:j * 512 + dq + 128],
                                pT[:, j * 512 + dq:j * 512 + dq + 128], tri_sb)
                    for j, kc, dq in subs:
                        nc.tensor.matmul(
                            ps_av[0:65, dq:512],
                            lhsT=v_sb[:, kc, h * 65:h * 65 + 65],
                            rhs=pT[:, j * 512 + dq:(j + 1) * 512],
                            start=(kc == 0), stop=(kc == nkc - 1))
                # softmax denominator: psum row 64 -> sbuf -> DRAM, replicate
                # across 64 partitions via a broadcast DMA read, then
                # approx-reciprocal
                lrow = rpool.tile([65, 512], f32, tag="lrow")
                nc.vector.tensor_copy(lrow[64:65, :], ps_av[64:65, :])
                ld = ldram.tile([1, 512], f32, tag="ld")
                nc.gpsimd.dma_start(out=ld, in_=lrow[64:65, :])
                lbc = rpool.tile([64, 512], f32, tag="lbc")
                nc.gpsimd.dma_start(out=lbc, in_=ld.to_broadcast([64, 512]))
                rb = rpool.tile([64, 512], f32, tag="rb")
                nc.vector.reciprocal_approx_fast(out=rb, in_=lbc)
                nc.vector.tensor_mul(mergedT[hr:hr + 64, hc, q0:q0 + 512],
                                     ps_av[0:64, :], rb)
                # PE filler work emitted mid-round
                if h == 3 and qt < QT - 1:
                    qkproj(qt + 1)
                if h == 5 and qt > 0:
                    outproj(qt - 1)
        outproj(QT - 1)


# revision 18
# speedup vs baseline: 1.0188x; 1.0188x over previous
"""Multi-head self-attention (LayerNorm + causal MHA + output projection) on 8 trn2 cores.

Sharding: core i handles (batch b = i//2, head-group g = i%2); each head-group
is 8 of the 16 heads (512 of 1024 feature dims). The output projection is
row-sharded: each core computes a full [1024, 2048] partial; the host sums the
two partials per batch (the "all-reduce") and adds the output bias.

On-chip layout (per core), everything transposed so the contraction dim sits on
partitions:
  xnT  [c=128x8, s=2048]  bf16   LayerNorm'd input, transposed via PE
  qT,kT [o=128x4, s=2048] bf16   Q^T (pre-scaled by 1/sqrt(dh)), K^T
  v    [s=128x16, 8*(64+1)] bf16 V in [s, d] layout + ones column per head
                                 (the ones column makes the AV matmul also
                                  produce the softmax denominator row)
  scoresT psum [k=128, q]  f32   -> exp on ACT -> pT bf16 (causal blocks only)
  av psum [65, 512]        f32   rows 0..63 = (P@V)^T head slice, row 64 = l
  mergedT [m=128x4, s=2048] bf16 (P@V)^T / l, heads stacked
  out  [o=1024, s=2048]    f32   woT @ mergedT partial, DMA'd out
"""

import numpy as np
import ml_dtypes
from contextlib import ExitStack

import concourse.bass as bass
import concourse.mybir as mybir
import concourse.tile as tile
from concourse import bacc
from concourse.bass_utils import run_bass_kernel_spmd

f32 = mybir.dt.float32
bf16 = mybir.dt.bfloat16
AF = mybir.ActivationFunctionType
ALU = mybir.AluOpType

B, S, D = 4, 2048, 1024
H, DH = 16, 64
N_CORES = 8
O = 512          # head dims per core (8 heads)
HL = 8           # heads per core
CC = 8           # c chunks (1024/128)
ST = 16          # s tiles of 128
QT = 4           # q tiles of 512
EPS = 1e-5


def emit(nc, tc, ctx):
    x_d = nc.dram_tensor("x", [S, D], bf16, kind="ExternalInput").ap()
    wq_d = nc.dram_tensor("wq", [128, CC, O], bf16, kind="ExternalInput").ap()
    wk_d = nc.dram_tensor("wk", [128, CC, O], bf16, kind="ExternalInput").ap()
    wv_d = nc.dram_tensor("wv", [128, CC, O], bf16, kind="ExternalInput").ap()
    wo_d = nc.dram_tensor("wo", [128, 4, D], bf16, kind="ExternalInput").ap()
    g2_d = nc.dram_tensor("g2", [128, CC], f32, kind="ExternalInput").ap()
    b2_d = nc.dram_tensor("b2", [128, CC], f32, kind="ExternalInput").ap()
    bq_d = nc.dram_tensor("bq2", [128, 4], f32, kind="ExternalInput").ap()
    bk_d = nc.dram_tensor("bk2", [128, 4], f32, kind="ExternalInput").ap()
    bvb_d = nc.dram_tensor("bvb", [128, O], f32, kind="ExternalInput").ap()
    tri_d = nc.dram_tensor("tri", [128, 128], bf16, kind="ExternalInput").ap()
    id_d = nc.dram_tensor("ident", [128, 128], bf16, kind="ExternalInput").ap()
    out_d = nc.dram_tensor("out", [D, S], f32, kind="ExternalOutput").ap()

    persist = ctx.enter_context(tc.tile_pool(name="persist", bufs=1))

    xnT = persist.tile([128, CC, S], bf16)
    qT = persist.tile([128, 4, S], bf16)
    kT = persist.tile([128, 4, S], bf16)
    v_sb = persist.tile([128, ST, HL * 65], bf16)
    mergedT = persist.tile([128, 4, S], bf16)

    wq_sb = persist.tile([128, CC, O], bf16)
    wk_sb = persist.tile([128, CC, O], bf16)
    wv_sb = persist.tile([128, CC, O], bf16)
    wo_sb = persist.tile([128, 4, D], bf16)
    g2_sb = persist.tile([128, CC], f32)
    b2_sb = persist.tile([128, CC], f32)
    bq_sb = persist.tile([128, 4], f32)
    bk_sb = persist.tile([128, 4], f32)
    bvb_sb = persist.tile([128, O], f32)
    tri_sb = persist.tile([128, 128], bf16)
    id_sb = persist.tile([128, 128], bf16)
    eps_sb = persist.tile([128, 1], f32)

    # small constants first; the x tiles stream in during stage A; weights
    # queue behind them (sync DMA queue is in-order) and arrive before stage B
    for dst, src in [(id_sb, id_d), (g2_sb, g2_d), (b2_sb, b2_d)]:
        nc.sync.dma_start(out=dst, in_=src)
    nc.vector.memset(eps_sb, EPS)
    # warm the ACT function tables before the LN chain needs them
    warm_sb = persist.tile([128, 1], f32)
    nc.scalar.activation(out=warm_sb, in_=eps_sb, func=AF.Sqrt)
    nc.scalar.activation(out=warm_sb, in_=eps_sb, func=AF.Exp)
    # ones columns of v (col 64 of each 65-wide head block)
    v4 = v_sb.rearrange("p st (h e) -> p st h e", e=65)
    nc.vector.memset(v4[:, :, :, 64:65], 1.0)

    # ---- Stage A: LayerNorm + transpose -> xnT, V projection interleaved ----
    xpool = ctx.enter_context(tc.tile_pool(name="xpool", bufs=5))
    xnpool = ctx.enter_context(tc.tile_pool(name="xnpool", bufs=8))
    spool = ctx.enter_context(tc.tile_pool(name="spool", bufs=8))

    # weights go on the scalar engine's DMA queue so the x tiles (sync queue)
    # are not blocked behind them at startup
    for dst, src_ in [(wv_sb, wv_d), (bvb_sb, bvb_d), (wq_sb, wq_d), (bq_sb, bq_d),
                      (wk_sb, wk_d), (bk_sb, bk_d), (tri_sb, tri_d), (wo_sb, wo_d)]:
        nc.scalar.dma_start(out=dst, in_=src_)
    bvb_v = bvb_sb.rearrange("p (h e) -> p h e", e=64)

    with tc.tile_pool(name="trpsum", bufs=3, space="PSUM") as trpsum, \
         tc.tile_pool(name="vpsum", bufs=3, space="PSUM") as vpsum:
        for stq in range(4):
            xn_grp = []
            for j in range(4):
                st = stq * 4 + j
                x_t = xpool.tile([128, D], bf16, tag="x_t")
                nc.sync.dma_start(out=x_t[:, 0:512],
                                  in_=x_d[st * 128:(st + 1) * 128, 0:512])
                nc.sync.dma_start(out=x_t[:, 512:1024],
                                  in_=x_d[st * 128:(st + 1) * 128, 512:1024])
                stats = spool.tile([128, 2, 6], f32, tag="stats")
                nc.vector.bn_stats(out=stats[:, 0, :], in_=x_t[:, 0:512])
                nc.vector.bn_stats(out=stats[:, 1, :], in_=x_t[:, 512:1024])
                mv = spool.tile([128, 2], f32, tag="mv")
                nc.vector.bn_aggr(out=mv, in_=stats)
                sq = spool.tile([128, 1], f32, tag="sq")
                nc.scalar.activation(out=sq, in_=mv[:, 1:2], func=AF.Sqrt, bias=eps_sb)
                rstd = spool.tile([128, 1], f32, tag="rstd")
                nc.vector.reciprocal(rstd, sq)
                nmr = spool.tile([128, 1], f32, tag="nmr")
                nc.vector.scalar_tensor_tensor(
                    out=nmr, in0=mv[:, 0:1], scalar=-1.0, in1=rstd,
                    op0=ALU.mult, op1=ALU.mult)
                xn_t = xnpool.tile([128, D], bf16, tag="xn_t")
                nc.scalar.activation(out=xn_t, in_=x_t, func=AF.Identity,
                                     bias=nmr, scale=rstd)
                xn_grp.append(xn_t)
            for cc in range(CC):
                ps_tr = trpsum.tile([128, 512], bf16, tag="ps_tr")
                for j in range(4):
                    nc.tensor.transpose(ps_tr[:, j * 128:(j + 1) * 128],
                                        xn_grp[j][:, cc * 128:(cc + 1) * 128], id_sb)
                nc.scalar.activation(
                    out=xnT[:, cc, stq * 512:(stq + 1) * 512], in_=ps_tr,
                    func=AF.Identity, scale=g2_sb[:, cc:cc + 1],
                    bias=b2_sb[:, cc:cc + 1])
            # V projection for this stq group (k rows stq*512 .. stq*512+512)
            for j in range(4):
                st = stq * 4 + j
                ps = vpsum.tile([128, 512], f32, tag="ps_v")
                for cc in range(CC):
                    nc.tensor.matmul(ps, lhsT=xnT[:, cc, st * 128:(st + 1) * 128],
                                     rhs=wv_sb[:, cc, :],
                                     start=(cc == 0), stop=(cc == CC - 1))
                nc.vector.scalar_tensor_tensor(
                    out=v4[:, st, :, 0:64],
                    in0=ps.rearrange("p (h e) -> p h e", e=64),
                    scalar=1.0, in1=bvb_v, op0=ALU.mult, op1=ALU.add)

    # ---- Stage C: QK projection per q/k-slice + attention rounds ----
    ppool = ctx.enter_context(tc.tile_pool(name="ppool", bufs=6))
    rpool = ctx.enter_context(tc.tile_pool(name="rpool", bufs=3))
    ldram = ctx.enter_context(tc.tile_pool(name="ldram", bufs=4, space="DRAM"))
    opool = ctx.enter_context(tc.tile_pool(name="opool", bufs=4))

    # qt-major rounds: round qt only needs Q/K columns for s < (qt+1)*512, so
    # the QK projection for slice nt is emitted inside round nt-1 (PE filler
    # while ACT runs exp), and the output projection for columns of round qt-1
    # interleaves likewise.  This keeps the PE dense so HAM stays unthrottled.
    with tc.tile_pool(name="spsum", bufs=2, space="PSUM") as spsum, \
         tc.tile_pool(name="avpsum", bufs=3, space="PSUM") as avpsum, \
         tc.tile_pool(name="qkpsum", bufs=1, space="PSUM") as qkpsum:

        def qkproj(nt):
            for w_sb, dst, b_sb in [(wq_sb, qT, bq_sb), (wk_sb, kT, bk_sb)]:
                for oc in range(4):
                    ps = qkpsum.tile([128, 512], f32, tag="ps_qk")
                    for cc in range(CC):
                        nc.tensor.matmul(
                            ps, lhsT=w_sb[:, cc, oc * 128:(oc + 1) * 128],
                            rhs=xnT[:, cc, nt * 512:(nt + 1) * 512],
                            start=(cc == 0), stop=(cc == CC - 1))
                    nc.vector.tensor_scalar_add(
                        out=dst[:, oc, nt * 512:(nt + 1) * 512], in0=ps,
                        scalar1=b_sb[:, oc:oc + 1])

        def outproj(nt):
            for oc8 in range(8):
                ps_o = avpsum.tile([128, 512], f32, tag="ps_av")
                for mc in range(4):
                    nc.tensor.matmul(
                        ps_o[0:128, :], lhsT=wo_sb[:, mc, oc8 * 128:(oc8 + 1) * 128],
                        rhs=mergedT[:, mc, nt * 512:(nt + 1) * 512],
                        start=(mc == 0), stop=(mc == 3))
                ob = opool.tile([128, 512], f32, tag="ob")
                nc.vector.tensor_copy(ob, ps_o[0:128, :])
                nc.sync.dma_start(
                    out=out_d[oc8 * 128:(oc8 + 1) * 128, nt * 512:(nt + 1) * 512],
                    in_=ob)

        qkproj(0)
        for qt in range(QT):
            q0 = qt * 512
            nkc = 4 * qt + 4
            for h in range(HL):
                hr = 64 * (h % 2)
                hc = h // 2
                ql = qT[hr:hr + 64, hc, :]
                kl = kT[hr:hr + 64, hc, :]
                ps_av = avpsum.tile([128, 512], f32, tag="ps_av")
                for kc2 in range(nkc // 2):
                    ps_s = spsum.tile([128, 1024], f32, tag="ps_s")
                    pT = ppool.tile([128, 1024], bf16, tag="pT")
                    subs = []
                    for j in range(2):
                        kc = 2 * kc2 + j
                        dq = max(0, (kc - 4 * qt) * 128)
                        nc.tensor.matmul(
                            ps_s[:, j * 512 + dq:(j + 1) * 512],
                            lhsT=kl[:, kc * 128:(kc + 1) * 128],
                            rhs=ql[:, q0 + dq:q0 + 512], start=True, stop=True)
                        subs.append((j, kc, dq))
                    # one wide exp per psum group; unwritten prefix columns of
                    # diagonal blocks hold stale psum - exp'd but never read
                    nc.scalar.activation(out=pT, in_=ps_s, func=AF.Exp)
                    for j, kc, dq in subs:
                        if kc >= 4 * qt:
                            nc.vector.tensor_mul(
                                pT[:, j * 512 + dq:j * 512 + dq + 128],
                                pT[:, j * 512 + dq:j * 512 + dq + 128], tri_sb)
                    for j, kc, dq in subs:
                        nc.tensor.matmul(
                            ps_av[0:65, dq:512],
                            lhsT=v_sb[:, kc, h * 65:h * 65 + 65],
                            rhs=pT[:, j * 512 + dq:(j + 1) * 512],
                            start=(kc == 0), stop=(kc == nkc - 1))
                # softmax denominator: psum row 64 -> sbuf -> DRAM, replicate
                # across 64 partitions via a broadcast DMA read, then
                # approx-reciprocal
                lrow = rpool.tile([65, 512], f32, tag="lrow")
                nc.vector.tensor_copy(lrow[64:65, :], ps_av[64:65, :])
                ld = ldram.tile([1, 512], f32, tag="ld")
                nc.sync.dma_start(out=ld, in_=lrow[64:65, :])
                lbc = rpool.tile([64, 512], f32, tag="lbc")
                nc.sync.dma_start(out=lbc, in_=ld.to_broadcast([64, 512]))
                rb = rpool.tile([64, 512], f32, tag="rb")
                nc.vector.reciprocal_approx_fast(out=rb, in_=lbc)
                nc.vector.tensor_mul(mergedT[hr:hr + 64, hc, q0:q0 + 512],
                                     ps_av[0:64, :], rb)
                # PE filler work emitted mid-round
                if h == 3 and qt < QT - 1:
                    qkproj(qt + 1)
                if h == 5 and qt > 0:
                    outproj(qt - 1)
        outproj(QT - 1)
